# revision 1
# baseline (speedup 1.0000x reference)
"""Trainium2 Bass kernel for NodeLevelPromptRefiner.

Computes, for N=100000 nodes across 8 NeuronCores (data-parallel over nodes):

    out = relu(concat([node_feats, graph_prompt[batch_idx]]) @ W1 + bias1) @ W2 + bias2

Algorithm (per core, 12500 nodes = 24 blocks x 512 + one 212-wide tail):
  * Host precomputes PW = graph_prompt @ W1[512:] + bias1  (the prompt half of
    layer 1 collapsed to one [1024, 512] matrix; exact per node since each node
    uses exactly one prompt row), then gathers it per node: pexp = PW[batch_idx].
  * On device the prompt term is pre-copied into PSUM (Vector-engine
    tensor_copy; GPSIMD cannot access PSUM) and the four layer-1 node matmuls
    accumulate on top (start=False), so the PE only does the 512-deep node
    contraction — no one-hot matmul.
  * Activations live feature-major on chip (x^T layout, host pre-transposes),
    so both layers are plain stationary-weight matmuls and no on-chip
    transposes are needed; output is stored feature-major fp16 and host
    transposes back.
  * Per-block packed DRAM layout ([NBLK, 128, 2048]) makes every input/output
    stream one DMA descriptor per block, spread across the sync (x), gpsimd
    (prompt) and scalar (output) DGE rings.
  * Matmul path runs in float16 (fp32 matmul is 4x slower on the PE; fp16
    keeps ~11-bit mantissas vs bf16's 8). PSUM accumulation stays fp32.
"""

import sys

if "/opt/trn_rl_repo" not in sys.path:
    sys.path.insert(0, "/opt/trn_rl_repo")

import numpy as np

P = 128          # partitions / chunk size
D = 512          # node & prompt feature dim
KC = D // P      # contraction chunks per layer
DC = D // P      # output chunks per layer
BLK = 512        # nodes per device block (one PSUM bank wide)
NCORES = 8
N_NODES = 100000
NSH = N_NODES // NCORES   # 12500 nodes per core
NBLK = (NSH + BLK - 1) // BLK  # 25
TAIL = NSH - (NBLK - 1) * BLK  # 212 valid nodes in the last block
NP = NBLK * BLK           # 12800 padded nodes per core
NG = 1024                 # number of graphs

_CACHED_NC = None


def _build_nc():
    import concourse.mybir as mybir
    import concourse.tile as tile
    from concourse import bacc

    f32 = mybir.dt.float32
    f16 = mybir.dt.float16
    AF = mybir.ActivationFunctionType

    nc = bacc.Bacc("TRN2", target_bir_lowering=False, debug=False)
    xblk = nc.dram_tensor("xblk", [NBLK, P, KC * BLK], f16, kind="ExternalInput").ap()
    pexp = nc.dram_tensor("pexp", [NBLK, P, DC * BLK], f16, kind="ExternalInput").ap()
    w1a = nc.dram_tensor("w1a", [D, D], f16, kind="ExternalInput").ap()
    w2 = nc.dram_tensor("w2", [D, D], f16, kind="ExternalInput").ap()
    bias2 = nc.dram_tensor("bias2", [D], f32, kind="ExternalInput").ap()
    outb = nc.dram_tensor("outb", [NBLK, P, DC * BLK], f16, kind="ExternalOutput").ap()

    w1a_r = w1a.rearrange("(kc p) (dc j) -> p kc dc j", p=P, j=P)
    w2_r = w2.rearrange("(kc p) (dc j) -> p kc dc j", p=P, j=P)
    bias2_r = bias2.rearrange("(dc p) -> p dc", p=P)

    with tile.TileContext(nc) as tc:
        with (
            tc.tile_pool(name="consts", bufs=1) as cp,
            # Input pools at depth 2, not 3: DMA queues round-robin packets
            # across every queued descriptor, so a third prefetched block
            # just dilutes block 0's arrival at startup (+~3us to first
            # real matmul). Depth 2 still prefetches one full block ahead
            # (2.9us of DMA vs 6.8us of compute per block).
            tc.tile_pool(name="xt", bufs=2) as xp,
            tc.tile_pool(name="pe", bufs=2) as pep,
            tc.tile_pool(name="h", bufs=2) as hp,
            tc.tile_pool(name="os", bufs=3) as osp,
            tc.tile_pool(name="ps", bufs=4, space="PSUM") as psp,
        ):
            # Startup-critical-path order: W1 chunk 0, then block 0's
            # activations, then the rest of the weights — so the first
            # matmul only waits on ~256KB, not the full weight set.
            w1s = cp.tile([P, KC, DC, P], f16)
            w2s = cp.tile([P, KC, DC, P], f16)
            b2s = cp.tile([P, DC], f32)

            # Weight loads stay on the sync ring, queued BEHIND block 0/1's
            # x loads: per-ring delivery is serial, so the critical block-0
            # bytes drain first. (Moving weights to another ring just makes
            # them compete for aggregate DMA bandwidth concurrently —
            # measured slower.)
            def load_consts(stage):
                if stage == 0:
                    for kc in range(1, KC):
                        nc.sync.dma_start(out=w1s[:, kc], in_=w1a_r[:, kc])
                    for kc in range(2):
                        nc.sync.dma_start(out=w2s[:, kc], in_=w2_r[:, kc])
                elif stage == 1:
                    for kc in range(2, KC):
                        nc.sync.dma_start(out=w2s[:, kc], in_=w2_r[:, kc])
                    nc.sync.dma_start(out=b2s[:], in_=bias2_r[:])

            # w1s[0] must stay on the sync ring: routing it via the scalar
            # (output) queue measured a catastrophic +33us serialization.
            nc.sync.dma_start(out=w1s[:, 0], in_=w1a_r[:, 0])

            # PE warm-up: dependency-free matmuls on memset tiles keep the
            # PE continuously busy (HAM clock ramp + DMA pipeline fill:
            # ~2.3MB of block 0/1 inputs and weights must land before real
            # work can stream gap-free, ~16-17us in). Empirically this
            # exact configuration — 18 warmups on the ps1 ring — beats
            # every tried variant (fewer warmups, ps2 ring, chunked or
            # re-ordered startup DMAs): any idle gap before or between
            # early matmuls resets the clock ramp and costs more than the
            # warmups themselves.
            warm_w = cp.tile([P, P], f16)
            nc.vector.memset(warm_w[:], 0.0)
            warm_x = cp.tile([P, BLK], f16)
            nc.vector.memset(warm_x[:], 0.0)
            for i in range(10):
                wp = psp.tile([P, BLK], f32, name=f"warm{i}", tag="ps1")
                nc.tensor.matmul(
                    wp[:], lhsT=warm_w[:], rhs=warm_x[:], start=True, stop=True
                )

            for b in range(NBLK):
                W = BLK if b < NBLK - 1 else TAIL
                xt = xp.tile([P, KC * BLK], f16)
                pw = pep.tile([P, DC * BLK], f16)
                # One packed descriptor per tile, one engine queue per tile.
                # Hard constraints measured on this DMA system: each ring is
                # a ~90GB/s FIFO; input loads on the scalar ring serialize
                # against stores (+33us); and a tile DMA-written from two
                # different engine queues poisons the whole schedule
                # (+41us). Within those, this layout is the startup floor.
                nc.sync.dma_start(out=xt[:], in_=xblk[b])
                nc.gpsimd.dma_start(out=pw[:], in_=pexp[b])
                if b <= 1:
                    load_consts(b)

                # Layer 1: psum <- pexp chunk (DVE pre-copy), then
                # h^T[dc] = relu(psum + sum_kc W1a[kc,dc].T @ x^T[kc])
                h = hp.tile([P, KC * BLK], f16)
                for dc in range(DC):
                    ps = psp.tile([P, BLK], f32, name=f"ps1_{b}_{dc}", tag="ps1")
                    # Pre-copy alternates DVE / ACT by dc parity: the two
                    # copies run concurrently (halves the serial pre-copy
                    # chain gating block 0) and balances the two busiest
                    # non-PE engines (~117us / ~128us vs 172us PE).
                    if dc % 2 == 0:
                        nc.vector.tensor_copy(
                            ps[:, :W], pw[:, dc * BLK : dc * BLK + W]
                        )
                    else:
                        nc.scalar.activation(
                            ps[:, :W],
                            pw[:, dc * BLK : dc * BLK + W],
                            AF.Copy,
                        )
                    for kc in range(KC):
                        nc.tensor.matmul(
                            ps[:, :W],
                            lhsT=w1s[:, kc, dc, :],
                            rhs=xt[:, kc * BLK : kc * BLK + W],
                            start=False,
                            stop=(kc == KC - 1),
                            skip_group_check=True,
                        )
                    nc.scalar.activation(
                        h[:, dc * BLK : dc * BLK + W], ps[:, :W], AF.Relu
                    )

                # Layer 2, kc-outer so PE can start as soon as relu chunk 0
                # lands: out^T[dc] = sum_kc W2[kc,dc].T @ h^T[kc] + bias2[dc]
                osb = osp.tile([P, DC * BLK], f16)
                ps2 = [
                    psp.tile([P, BLK], f32, name=f"ps2_{b}_{i}", tag="ps2")
                    for i in range(DC)
                ]
                # kc-outer: PE starts L2 as soon as relu chunk 0 lands.
                # Last block runs dc-outer instead, so its first output
                # chunks store while the rest still compute (shorter tail).
                if b < NBLK - 1:
                    order = [(kc, dc) for kc in range(KC) for dc in range(DC)]
                else:
                    order = [(kc, dc) for dc in range(DC) for kc in range(KC)]
                for kc, dc in order:
                    nc.tensor.matmul(
                        ps2[dc][:, :W],
                        lhsT=w2s[:, kc, dc, :],
                        rhs=h[:, kc * BLK : kc * BLK + W],
                        start=(kc == 0),
                        stop=(kc == KC - 1),
                        skip_group_check=True,
                    )
                    if kc == KC - 1:
                        # bias2 add on DVE (ACT is busier); packed per-block
                        # output DMA on the ACT HWDGE ring.
                        nc.vector.tensor_scalar_add(
                            osb[:, dc * BLK : dc * BLK + W],
                            ps2[dc][:, :W],
                            b2s[:, dc : dc + 1],
                        )
                        if b < NBLK - 1:
                            if dc == DC - 1:
                                nc.scalar.dma_start(out=outb[b], in_=osb[:])
                        else:
                            nc.scalar.dma_start(
                                out=outb[b, :, dc * BLK : dc * BLK + W],
                                in_=osb[:, dc * BLK : dc * BLK + W],
                            )

    nc.compile()
    return nc


def _get_nc():
    global _CACHED_NC
    if _CACHED_NC is None:
        _CACHED_NC = _build_nc()
    return _CACHED_NC


def _pack_blocks(arr_t):
    """[NSH(+pad), D] f16 -> [NBLK, P, (D//P)*BLK] block-packed layout."""
    out = np.zeros((NP, D), np.float16)
    out[: arr_t.shape[0]] = arr_t
    return np.ascontiguousarray(
        out.reshape(NBLK, BLK, D // P, P).transpose(0, 3, 2, 1)
    ).reshape(NBLK, P, (D // P) * BLK)


def _prep_core_inputs(node_feats, batch_idx, PW16, core):
    """Build the per-core device tensors (xblk, pexp)."""
    sh = slice(core * NSH, (core + 1) * NSH)
    x = node_feats[sh].astype(np.float16)
    bi = batch_idx[sh]
    # _pack_blocks wants [n, f] with f = chunk*128 + p; its reshape is
    # [NBLK, BLK, KC, P] -> [NBLK, P, KC, BLK], i.e. out[b,p,kc*BLK+j]
    # = in[b*BLK+j, kc*128+p]. That matches the device-side rearrange.
    return {"xblk": _pack_blocks(x), "pexp": _pack_blocks(PW16[bi])}


def _run(inputs, trace=False, trace_cores=None, repeats=1):
    """Full pipeline: host prep -> 8-core SPMD run -> gather.

    Returns (output [100000, 512] f32, BassKernelResults). With repeats>1,
    reruns the device step and returns the run with min exec_time_ns
    (exec times of all runs in res.all_exec_times_ns)."""
    from concourse.bass_utils import run_bass_kernel_spmd

    node_feats = np.asarray(inputs["node_feats"], np.float32)
    graph_prompt = np.asarray(inputs["graph_prompt"], np.float32)
    batch_idx = np.asarray(inputs["batch_idx"]).astype(np.int64)
    W1 = np.asarray(inputs["W1"], np.float32)
    bias1 = np.asarray(inputs["bias1"], np.float32)
    W2 = np.asarray(inputs["W2"], np.float32)
    bias2 = np.asarray(inputs["bias2"], np.float32)

    # Prompt half of layer 1, collapsed per graph (in float64 for accuracy).
    PW = (
        graph_prompt.astype(np.float64) @ W1[D:].astype(np.float64)
        + bias1.astype(np.float64)
    ).astype(np.float32)
    PW16 = PW.astype(np.float16)

    w1a = np.ascontiguousarray(W1[:D]).astype(np.float16)
    w2m = W2.astype(np.float16)

    in_maps = []
    for c in range(NCORES):
        m = _prep_core_inputs(node_feats, batch_idx, PW16, c)
        m["w1a"] = w1a
        m["w2"] = w2m
        m["bias2"] = bias2
        in_maps.append(m)

    nc = _get_nc()
    kw = {}
    if trace:
        kw["trace"] = True
        if trace_cores is not None:
            kw["trace_cores"] = trace_cores
    # First execution in a fresh process is unreliable on this stack (reads
    # can race initial input upload; observed garbage/NaN on run 0 only, with
    # runs 1+ always correct). Always discard a throwaway first execution.
    run_bass_kernel_spmd(nc, in_maps, core_ids=list(range(NCORES)))
    res = run_bass_kernel_spmd(nc, in_maps, core_ids=list(range(NCORES)), **kw)
    times = [res.exec_time_ns]
    for _ in range(repeats - 1):
        r2 = run_bass_kernel_spmd(nc, in_maps, core_ids=list(range(NCORES)), **kw)
        times.append(r2.exec_time_ns)
        if r2.exec_time_ns is not None and (
            res.exec_time_ns is None or r2.exec_time_ns < res.exec_time_ns
        ):
            res = r2
    res.all_exec_times_ns = times

    def gather(r):
        o = np.empty((N_NODES, D), np.float32)
        for c in range(NCORES):
            ob = r.results[c]["outb"]  # [NBLK, P, DC*BLK] f16
            full = (
                ob.reshape(NBLK, P, DC, BLK)
                .transpose(0, 3, 2, 1)
                .reshape(NP, D)
            )
            o[c * NSH : (c + 1) * NSH] = full[:NSH].astype(np.float32)
        return o

    out = gather(res)
    # Plausibility net: legit outputs are O(1); NaN or huge values mean a
    # corrupted execution — retry once.
    if np.isnan(out).any() or np.abs(out).max() > 100.0:
        res = run_bass_kernel_spmd(nc, in_maps, core_ids=list(range(NCORES)), **kw)
        out = gather(res)
    return out, res


def kernel(**inputs):
    return _run(inputs)[0]



# revision 6
# speedup vs baseline: 1.2781x; 1.2781x over previous
"""Trainium2 Bass kernel for NodeLevelPromptRefiner.

Computes, for N=100000 nodes across 8 NeuronCores (data-parallel over nodes):

    out = relu(concat([node_feats, graph_prompt[batch_idx]]) @ W1 + bias1) @ W2 + bias2

Algorithm (per core, 12500 nodes = 24 blocks x 512 + one 212-wide tail):
  * Host precomputes PW = graph_prompt @ W1[512:] + bias1  (the prompt half of
    layer 1 collapsed to one [1024, 512] matrix; exact per node since each node
    uses exactly one prompt row), then gathers it per node: pexp = PW[batch_idx].
  * On device the prompt term is pre-copied into PSUM (Vector/Scalar-engine
    copy; GPSIMD cannot access PSUM) and the layer-1 node matmuls accumulate
    on top (start=False), so the PE only does the 512-deep node contraction.
  * Mixed-precision layer 1: the first half of the node contraction (k=0..256)
    runs as ONE fp8-e4m3 DoubleRow matmul per output chunk (2 MACs/cell/cycle,
    K=256 per pass); the second half (k=256..512) stays fp16. Layer 2 is all
    fp16. This cuts PE cycles ~11% while keeping rel-l2 error ~1.75e-2
    (measured in exact-dtype simulation; fp8 on more of the network busts the
    2e-2 budget - full L1 fp8 = 2.5e-2, full fp8 = 4.5e-2).
  * fp8 scaling: x goes into e4m3 UNSCALED (randn range fits; subnormal region
    is negligible); W1a is scaled x1024 so its +-0.031 entries leave e4m3's
    subnormal range. The whole L1 PSUM therefore carries 1024x values (pexp
    and the fp16 W chunks are pre-scaled x1024 on host) and the relu
    activation divides back by exactly 2^-10 via its scale parameter (relu is
    positively homogeneous, scale is a power of two => exact).
  * Activations live feature-major on chip (x^T layout, host pre-transposes),
    so both layers are plain stationary-weight matmuls; output is stored
    feature-major fp16 and host transposes back.
  * DMA: per-block packed layouts, one descriptor per tile per queue. Rings:
    sync = xt8 + all weights, vector = xt16, gpsimd = pexp (block 0 split into
    per-dc descriptors so the first PSUM pre-copy isn't gated on the full
    512KB), scalar(ACT) = outputs. Layer-1 DoubleRow passes are grouped before
    the fp16 passes so block 0 can start on the small xt8 stream (128KB)
    before xt16 (256KB) lands.
  * PSUM accumulation stays fp32 everywhere.
"""

import sys

if "/opt/trn_rl_repo" not in sys.path:
    sys.path.insert(0, "/opt/trn_rl_repo")

import numpy as np
import ml_dtypes

F8NP = ml_dtypes.float8_e4m3  # TRN float8e4: max +-240

P = 128          # partitions / chunk size
D = 512          # node & prompt feature dim
KC = D // P      # contraction chunks per layer (4)
DC = D // P      # output chunks per layer (4)
BLK = 512        # nodes per device block (one PSUM bank wide)
NCORES = 8
N_NODES = 100000
NSH = N_NODES // NCORES   # 12500 nodes per core
NBLK = (NSH + BLK - 1) // BLK  # 25
TAIL = NSH - (NBLK - 1) * BLK  # 212 valid nodes in the last block
NP = NBLK * BLK           # 12800 padded nodes per core
NG = 1024                 # number of graphs

WS = 1024.0               # weight scale for layer 1 (power of two, exact)
N_WARM = 4                # PE warmup matmuls (HAM ramp + DMA pipeline fill)

_CACHED_NC = None


def _build_nc():
    import concourse.mybir as mybir
    import concourse.tile as tile
    from concourse import bacc

    f32 = mybir.dt.float32
    f16 = mybir.dt.float16
    f8 = mybir.dt.float8e4
    AF = mybir.ActivationFunctionType
    DR = mybir.MatmulPerfMode.DoubleRow

    nc = bacc.Bacc("TRN2", target_bir_lowering=False, debug=False)
    # fp8 half of x: [b, p, i*BLK+j] = x[b*BLK+j, i*128+p], i in {0,1}
    xblk8 = nc.dram_tensor("xblk8", [NBLK, P, 2 * BLK], f8, kind="ExternalInput").ap()
    # fp16 half of x: [b, p, i*BLK+j] = x[b*BLK+j, (2+i)*128+p]
    xblk16 = nc.dram_tensor("xblk16", [NBLK, P, 2 * BLK], f16, kind="ExternalInput").ap()
    pexp = nc.dram_tensor("pexp", [NBLK, P, DC * BLK], f16, kind="ExternalInput").ap()
    # w1a8[p, i*512 + dc*128+m] = e4m3(1024*W1a[i*128+p, dc*128+m])
    w1a8 = nc.dram_tensor("w1a8", [P, 2 * D], f8, kind="ExternalInput").ap()
    # w1a16: k-chunks 2,3 of W1a, x1024, fp16, [256, 512]
    w1a16 = nc.dram_tensor("w1a16", [2 * P, D], f16, kind="ExternalInput").ap()
    w2 = nc.dram_tensor("w2", [D, D], f16, kind="ExternalInput").ap()
    bias2 = nc.dram_tensor("bias2", [D], f32, kind="ExternalInput").ap()
    outb = nc.dram_tensor("outb", [NBLK, P, DC * BLK], f16, kind="ExternalOutput").ap()

    w1a8_r = w1a8.rearrange("p (i dc m) -> p i dc m", i=2, dc=DC, m=P)
    w1a16_r = w1a16.rearrange("(kc p) (dc m) -> p kc dc m", p=P, m=P)
    w2_r = w2.rearrange("(kc p) (dc j) -> p kc dc j", p=P, j=P)
    bias2_r = bias2.rearrange("(dc p) -> p dc", p=P)
    pexp_d = pexp.rearrange("b p (dc j) -> b p dc j", dc=DC)

    with tile.TileContext(nc) as tc:
        with (
            tc.tile_pool(name="consts", bufs=1) as cp,
            # Input pools at depth 2: DMA queues round-robin packets across
            # every queued descriptor, so deeper prefetch just dilutes block
            # 0's arrival at startup.
            tc.tile_pool(name="x8", bufs=2) as xp8,
            tc.tile_pool(name="x16", bufs=2) as xp16,
            tc.tile_pool(name="pe", bufs=2) as pep,
            tc.tile_pool(name="h", bufs=2) as hp,
            tc.tile_pool(name="os", bufs=3) as osp,
            tc.tile_pool(name="ps", bufs=4, space="PSUM") as psp,
        ):
            w1s8 = cp.tile([P, 2, DC, P], f8)
            w1s16 = cp.tile([P, 2, DC, P], f16)
            w2s = cp.tile([P, KC, DC, P], f16)
            b2s = cp.tile([P, DC], f32)

            # Startup-critical order on the sync ring (per-ring delivery is
            # serial): fp8 L1 weights (128KB) first, then block 0/1 x-streams
            # queue in the loop, then the remaining weights behind them.
            nc.sync.dma_start(out=w1s8[:], in_=w1a8_r[:])

            def load_consts(stage):
                if stage == 0:
                    for kc in range(2):
                        nc.sync.dma_start(out=w1s16[:, kc], in_=w1a16_r[:, kc])
                elif stage == 1:
                    for kc in range(KC):
                        nc.sync.dma_start(out=w2s[:, kc], in_=w2_r[:, kc])
                    nc.sync.dma_start(out=b2s[:], in_=bias2_r[:])

            # PE warm-up: dependency-free matmuls on memset tiles cover the
            # HAM clock ramp + the DMA fill for block 0's first inputs.
            warm_w = cp.tile([P, P], f16)
            nc.vector.memset(warm_w[:], 0.0)
            warm_x = cp.tile([P, BLK], f16)
            nc.vector.memset(warm_x[:], 0.0)
            for i in range(N_WARM):
                wp = psp.tile([P, BLK], f32, name=f"warm{i}", tag="ps1")
                nc.tensor.matmul(
                    wp[:], lhsT=warm_w[:], rhs=warm_x[:], start=True, stop=True
                )

            xblk16_r = xblk16.rearrange("b p (kc j) -> b p kc j", kc=2)
            for b in range(NBLK):
                W = BLK if b < NBLK - 1 else TAIL
                xt8 = xp8.tile([P, 2, BLK], f8)
                # per-kc tiles/descriptors: the layer-1 fp16 phases can start
                # as soon as their own 128KB chunk lands, not the full 256KB.
                xt16a = xp16.tile([P, BLK], f16, name=f"x16a_{b}", tag="x16a")
                xt16b = xp16.tile([P, BLK], f16, name=f"x16b_{b}", tag="x16b")
                pw = pep.tile([P, DC, BLK], f16)
                nc.sync.dma_start(out=xt8[:], in_=xblk8[b])
                nc.sync.dma_start(out=xt16a[:], in_=xblk16_r[b, :, 0])
                nc.sync.dma_start(out=xt16b[:], in_=xblk16_r[b, :, 1])
                if b == 0:
                    # Split block 0's prompt load per-dc so the first PSUM
                    # pre-copy is gated on 128KB, not 512KB.
                    for dc in range(DC):
                        nc.gpsimd.dma_start(out=pw[:, dc], in_=pexp_d[b, :, dc])
                else:
                    nc.gpsimd.dma_start(out=pw[:], in_=pexp_d[b])
                if b <= 1:
                    load_consts(b)

                # Layer 1: psum <- pexp chunk (pre-copy, alternating DVE/ACT),
                # then one fp8 DoubleRow pass (k=0..256) + two fp16 passes
                # (k=256..512) accumulate on top. DoubleRow passes for all dc
                # are grouped first: they only need the small xt8 stream.
                h = hp.tile([P, KC * BLK], f16)
                ps = []
                for dc in range(DC):
                    p_ = psp.tile([P, BLK], f32, name=f"ps1_{b}_{dc}", tag="ps1")
                    ps.append(p_)
                    if dc % 2 == 0:
                        nc.vector.tensor_copy(p_[:, :W], pw[:, dc, :W])
                    else:
                        nc.scalar.activation(p_[:, :W], pw[:, dc, :W], AF.Copy)
                    nc.tensor.matmul(
                        p_[:, :W],
                        lhsT=w1s8[:, :, dc, :],
                        rhs=xt8[:, :, :W],
                        start=False,
                        stop=False,
                        perf_mode=DR,
                        skip_group_check=True,
                    )
                for kc, xt16 in ((0, xt16a), (1, xt16b)):
                    for dc in range(DC):
                        nc.tensor.matmul(
                            ps[dc][:, :W],
                            lhsT=w1s16[:, kc, dc, :],
                            rhs=xt16[:, :W],
                            start=False,
                            stop=(kc == 1),
                            skip_group_check=True,
                        )
                        if kc == 1:
                            # exact un-scale of the x1024 layer-1 weights
                            nc.scalar.activation(
                                h[:, dc * BLK : dc * BLK + W],
                                ps[dc][:, :W],
                                AF.Relu,
                                scale=1.0 / WS,
                            )

                # Layer 2, kc-outer so PE starts as soon as relu chunk 0
                # lands. Last block runs dc-outer so its first output chunks
                # store while the rest still compute (shorter tail).
                osb = osp.tile([P, DC * BLK], f16)
                ps2 = [
                    psp.tile([P, BLK], f32, name=f"ps2_{b}_{i}", tag="ps2")
                    for i in range(DC)
                ]
                if b < NBLK - 1:
                    order = [(kc, dc) for kc in range(KC) for dc in range(DC)]
                else:
                    order = [(kc, dc) for dc in range(DC) for kc in range(KC)]
                for kc, dc in order:
                    nc.tensor.matmul(
                        ps2[dc][:, :W],
                        lhsT=w2s[:, kc, dc, :],
                        rhs=h[:, kc * BLK : kc * BLK + W],
                        start=(kc == 0),
                        stop=(kc == KC - 1),
                        skip_group_check=True,
                    )
                    if kc == KC - 1:
                        # bias2 add; packed per-block output DMA on the ACT
                        # ring. Last block: alternate DVE/ACT for the adds and
                        # split the per-dc stores across ACT + gpsimd rings to
                        # shorten the drain.
                        if b < NBLK - 1:
                            nc.vector.tensor_scalar_add(
                                osb[:, dc * BLK : dc * BLK + W],
                                ps2[dc][:, :W],
                                b2s[:, dc : dc + 1],
                            )
                            if dc == DC - 1:
                                nc.scalar.dma_start(out=outb[b], in_=osb[:])
                        else:
                            if dc % 2 == 0:
                                nc.vector.tensor_scalar_add(
                                    osb[:, dc * BLK : dc * BLK + W],
                                    ps2[dc][:, :W],
                                    b2s[:, dc : dc + 1],
                                )
                            else:
                                nc.scalar.activation(
                                    osb[:, dc * BLK : dc * BLK + W],
                                    ps2[dc][:, :W],
                                    AF.Identity,
                                    bias=b2s[:, dc : dc + 1],
                                )
                            eng = nc.scalar if dc % 2 == 0 else nc.gpsimd
                            eng.dma_start(
                                out=outb[b, :, dc * BLK : dc * BLK + W],
                                in_=osb[:, dc * BLK : dc * BLK + W],
                            )

    nc.compile()
    return nc


def _get_nc():
    global _CACHED_NC
    if _CACHED_NC is None:
        _CACHED_NC = _build_nc()
    return _CACHED_NC


def _pack_half(arr, k0):
    """[NSH, D] -> [NBLK, P, 2*BLK] taking k-chunks k0, k0+1.

    out[b, p, i*BLK + j] = arr[b*BLK + j, (k0+i)*128 + p]
    """
    n = arr.shape[0]
    out = np.zeros((NP, 2 * P), arr.dtype)
    out[:n] = arr[:, k0 * P : (k0 + 2) * P]
    return np.ascontiguousarray(
        out.reshape(NBLK, BLK, 2, P).transpose(0, 3, 2, 1)
    ).reshape(NBLK, P, 2 * BLK)


def _pack_blocks(arr_t):
    """[NSH(+pad), D] -> [NBLK, P, (D//P)*BLK] block-packed layout."""
    out = np.zeros((NP, D), arr_t.dtype)
    out[: arr_t.shape[0]] = arr_t
    return np.ascontiguousarray(
        out.reshape(NBLK, BLK, D // P, P).transpose(0, 3, 2, 1)
    ).reshape(NBLK, P, (D // P) * BLK)


def _prep_core_inputs(node_feats, batch_idx, PW16, core):
    """Build the per-core device tensors (xblk8, xblk16, pexp)."""
    sh = slice(core * NSH, (core + 1) * NSH)
    x = node_feats[sh]
    bi = batch_idx[sh]
    x8 = np.clip(x, -240.0, 240.0).astype(F8NP)
    return {
        "xblk8": _pack_half(x8, 0),
        "xblk16": _pack_half(x.astype(np.float16), 2),
        "pexp": _pack_blocks(PW16[bi]),
    }


def _run(inputs, trace=False, trace_cores=None, repeats=1):
    """Full pipeline: host prep -> 8-core SPMD run -> gather.

    Returns (output [100000, 512] f32, BassKernelResults). With repeats>1,
    reruns the device step and returns the run with min exec_time_ns
    (exec times of all runs in res.all_exec_times_ns)."""
    from concourse.bass_utils import run_bass_kernel_spmd

    node_feats = np.asarray(inputs["node_feats"], np.float32)
    graph_prompt = np.asarray(inputs["graph_prompt"], np.float32)
    batch_idx = np.asarray(inputs["batch_idx"]).astype(np.int64)
    W1 = np.asarray(inputs["W1"], np.float32)
    bias1 = np.asarray(inputs["bias1"], np.float32)
    W2 = np.asarray(inputs["W2"], np.float32)
    bias2 = np.asarray(inputs["bias2"], np.float32)

    # Prompt half of layer 1, collapsed per graph (in float64 for accuracy),
    # pre-scaled x1024 to match the scaled layer-1 weights.
    PW = (
        graph_prompt.astype(np.float64) @ W1[D:].astype(np.float64)
        + bias1.astype(np.float64)
    ).astype(np.float32)
    PW16 = (PW * np.float32(WS)).astype(np.float16)

    W1a = np.ascontiguousarray(W1[:D])
    w1a8 = np.ascontiguousarray(
        np.clip(W1a[: 2 * P] * np.float32(WS), -240, 240)
        .astype(F8NP)
        .reshape(2, P, DC * P)
        .transpose(1, 0, 2)
    ).reshape(P, 2 * D)
    w1a16 = (W1a[2 * P :] * np.float32(WS)).astype(np.float16)
    w2m = W2.astype(np.float16)

    in_maps = []
    for c in range(NCORES):
        m = _prep_core_inputs(node_feats, batch_idx, PW16, c)
        m["w1a8"] = w1a8
        m["w1a16"] = w1a16
        m["w2"] = w2m
        m["bias2"] = bias2
        in_maps.append(m)

    nc = _get_nc()
    kw = {}
    if trace:
        kw["trace"] = True
        if trace_cores is not None:
            kw["trace_cores"] = trace_cores
    # First execution in a fresh process is unreliable on this stack (reads
    # can race initial input upload; observed garbage/NaN on run 0 only, with
    # runs 1+ always correct). Always discard a throwaway first execution.
    run_bass_kernel_spmd(nc, in_maps, core_ids=list(range(NCORES)))
    res = run_bass_kernel_spmd(nc, in_maps, core_ids=list(range(NCORES)), **kw)
    times = [res.exec_time_ns]
    for _ in range(repeats - 1):
        r2 = run_bass_kernel_spmd(nc, in_maps, core_ids=list(range(NCORES)), **kw)
        times.append(r2.exec_time_ns)
        if r2.exec_time_ns is not None and (
            res.exec_time_ns is None or r2.exec_time_ns < res.exec_time_ns
        ):
            res = r2
    res.all_exec_times_ns = times

    def gather(r):
        o = np.empty((N_NODES, D), np.float32)
        for c in range(NCORES):
            ob = r.results[c]["outb"]  # [NBLK, P, DC*BLK] f16
            full = (
                ob.reshape(NBLK, P, DC, BLK)
                .transpose(0, 3, 2, 1)
                .reshape(NP, D)
            )
            o[c * NSH : (c + 1) * NSH] = full[:NSH].astype(np.float32)
        return o

    out = gather(res)
    # Plausibility net: legit outputs are O(1); NaN or huge values mean a
    # corrupted execution - retry once.
    if np.isnan(out).any() or np.abs(out).max() > 100.0:
        res = run_bass_kernel_spmd(nc, in_maps, core_ids=list(range(NCORES)), **kw)
        out = gather(res)
    return out, res


def kernel(**inputs):
    return _run(inputs)[0]
